# revision 1
# baseline (speedup 1.0000x reference)
"""nn_AttnProcess_69355131896244 — Trainium2 kernel.

Strategy: pure data-parallel over batch B=8 across the 8 NeuronCores
(one batch element per core), exactly as the sharding hint suggests.
All window partitioning, per-window attention, LePE conv, and SASA
super-token attention are batch-independent; weights are replicated.

The per-core program is compiled for the NeuronCore via the PJRT
backend (jax.pmap over the 8 cores); inputs are sharded on the batch
axis, weights broadcast, and the full (8, 4096, 192) output gathered
back to the host.
"""
import numpy as np

H = W = 64
C_DIM = 192
HEADS = 6
TOK = 4
KS2 = 9
WEIGHT_FACTOR = 0.1

_WEIGHT_NAMES = [
    "w_qkv", "w_proj", "b_proj", "cw0", "cb0", "cw1", "cb1", "cw2", "cb2",
    "w_sqkv", "w_sproj", "b_sproj",
]


def _build_model(jnp, lax):
    """Return a function (x_slice, *weights) -> out_slice, pure jax."""
    _EYE = np.eye(KS2, dtype=np.float32).reshape(KS2, 1, 3, 3)
    _FOLD_W = _EYE[:, :, ::-1, ::-1].transpose(1, 0, 2, 3).copy()
    EYE = jnp.asarray(_EYE)
    FOLD_W = jnp.asarray(_FOLD_W)
    DN = ("NCHW", "OIHW", "NCHW")

    def _dwconv3(x, w, b):
        y = lax.conv_general_dilated(x, w, (1, 1), ((1, 1), (1, 1)),
                                     feature_group_count=x.shape[1],
                                     dimension_numbers=DN)
        return y + b[None, :, None, None]

    def _unfold(x):
        B, C, h, w = x.shape
        y = lax.conv_general_dilated(x.reshape(B * C, 1, h, w), EYE, (1, 1),
                                     ((1, 1), (1, 1)), dimension_numbers=DN)
        return y.reshape(B, C * KS2, h * w)

    def _fold(x):
        return lax.conv_general_dilated(x, FOLD_W, (1, 1), ((1, 1), (1, 1)),
                                        dimension_numbers=DN)

    def _cssa(qkv, cw, cb, hs, ws):
        import jax
        q, k, v = qkv[0], qkv[1], qkv[2]
        B, N, Cc = q.shape
        heads = HEADS // 3
        hd = Cc // heads
        scale = hd ** (-0.5)
        nh, nw = H // hs, W // ws

        def to_win(t):
            ti = t.transpose(0, 2, 1).reshape(B, Cc, H, W)
            tw = ti.reshape(B, Cc, nh, hs, nw, ws).transpose(0, 2, 4, 3, 5, 1)
            tw = tw.reshape(-1, hs * ws, Cc)
            return tw.reshape(-1, hs * ws, heads, hd).transpose(0, 2, 1, 3)

        qw = to_win(q) * scale
        kw = to_win(k)

        vi = v.transpose(0, 2, 1).reshape(B, Cc, H, W)
        vwin = vi.reshape(B, Cc, nh, hs, nw, ws).transpose(0, 2, 4, 1, 3, 5)
        vwin = vwin.reshape(-1, Cc, hs, ws)
        lepe = _dwconv3(vwin, cw, cb)
        lepe = lepe.reshape(-1, heads, hd, hs * ws).transpose(0, 1, 3, 2)
        vw = vwin.reshape(-1, heads, hd, hs * ws).transpose(0, 1, 3, 2)

        attn = jax.nn.softmax(jnp.einsum("whid,whjd->whij", qw, kw), axis=-1)
        o = jnp.einsum("whij,whjd->whid", attn, vw) + lepe
        o = o.transpose(0, 2, 1, 3).reshape(-1, hs * ws, Cc)
        o = o.reshape(B, nh, nw, hs, ws, Cc).transpose(0, 1, 3, 2, 4, 5)
        o = o.reshape(B, H, W, Cc)
        return o.reshape(B, N, Cc)

    def _sasa(x, w_sqkv, w_sproj, b_sproj):
        import jax
        B, N, C = x.shape
        xi = x.transpose(0, 2, 1).reshape(B, C, H, W)
        hh, ww = H // TOK, W // TOK
        blocks = xi.reshape(B, C, hh, TOK, ww, TOK)
        wf = blocks.mean(axis=(3, 5))
        pix = blocks.transpose(0, 2, 4, 3, 5, 1).reshape(B, hh * ww, TOK * TOK, C)

        wfu = _unfold(wf).transpose(0, 2, 1).reshape(B, hh * ww, C, KS2)
        aff = jax.nn.softmax(
            jnp.einsum("bwpc,bwck->bwpk", pix, wfu) * (C ** -0.5), axis=-1)

        aff_sum = aff.sum(2).transpose(0, 2, 1).reshape(B, KS2, hh, ww)
        aff_sum = _fold(aff_sum)

        wf2 = jnp.einsum("bwpc,bwpk->bwck", pix, aff)
        wf2 = wf2.transpose(0, 2, 3, 1).reshape(B * C, KS2, hh, ww)
        wf2 = _fold(wf2).reshape(B, C, hh, ww)
        wf2 = wf2 / (lax.stop_gradient(aff_sum) + 1e-12)

        hd = C // HEADS
        qkv = jnp.einsum("oc,bchw->bohw", w_sqkv, wf2).reshape(B, HEADS, 3 * hd, hh * ww)
        q, k, v = qkv[:, :, :hd], qkv[:, :, hd:2 * hd], qkv[:, :, 2 * hd:]
        attn = jnp.einsum("bhdn,bhdm->bhnm", k, q) * (hd ** -0.5)
        attn = jax.nn.softmax(attn, axis=-2)
        r = jnp.einsum("bhdn,bhnm->bhdm", v, attn).reshape(B, C, hh, ww)
        r = jnp.einsum("oc,bchw->bohw", w_sproj, r) + b_sproj[None, :, None, None]

        ru = _unfold(r).transpose(0, 2, 1).reshape(B, hh * ww, C, KS2)
        pix2 = jnp.einsum("bwck,bwpk->bwcp", ru, aff)
        pix2 = pix2.reshape(B, hh, ww, C, TOK, TOK).transpose(0, 3, 1, 4, 2, 5)
        pix2 = pix2.reshape(B, C, H, W)
        return pix2.reshape(B, H * W, C)

    def model(x, w_qkv, w_proj, b_proj, cw0, cb0, cw1, cb1, cw2, cb2,
              w_sqkv, w_sproj, b_sproj):
        B, N, C = x.shape
        c3 = C // 3
        qkv = (x @ w_qkv).reshape(B, N, 3, C).transpose(2, 0, 1, 3)
        x_h = _cssa(qkv[..., :c3], cw0, cb0, H, 1)
        x_s = _cssa(qkv[..., c3:2 * c3], cw2, cb2, H // 2, W // 2)
        x_v = _cssa(qkv[..., 2 * c3:], cw1, cb1, 1, W)
        cssa_x = jnp.concatenate([x_h, x_s, x_v], axis=2)
        cssa_x = cssa_x.reshape(B, N, 4, C // 4).transpose(0, 1, 3, 2).reshape(B, N, C)
        sasa_x = _sasa(x, w_sqkv, w_sproj, b_sproj)
        out = (cssa_x + WEIGHT_FACTOR * sasa_x) @ w_proj + b_proj
        return out

    return model


_CACHE = {}


def _get_pmapped():
    if "fn" in _CACHE:
        return _CACHE["fn"]
    import jax
    import jax.numpy as jnp
    from jax import lax

    model = _build_model(jnp, lax)
    devs = jax.devices()
    n = min(8, len(devs))
    # batch-parallel: axis 0 of x sharded one element per core; weights broadcast
    fn = jax.pmap(
        model,
        axis_name="b",
        in_axes=(0,) + (None,) * 12,
        devices=devs[:n],
    )
    _CACHE["fn"] = fn
    return fn


def kernel(**inputs) -> np.ndarray:
    x = np.asarray(inputs["x"], dtype=np.float32)
    ws = [np.asarray(inputs[k], dtype=np.float32) for k in _WEIGHT_NAMES]
    B = x.shape[0]
    # one batch element per core: (8, 4096, 192) -> 8 x (1, 4096, 192)
    xs = x.reshape(B, 1, *x.shape[1:])
    fn = _get_pmapped()
    out = fn(xs, *ws)            # (8, 1, N, C)
    out = np.asarray(out)
    return out.reshape(B, *out.shape[2:]).astype(np.float32)


if __name__ == "__main__":
    rng = np.random.default_rng(0)
    ins = {
        "x": rng.standard_normal((8, 4096, 192), dtype=np.float32),
        "w_qkv": (rng.standard_normal((192, 576)) * 0.02).astype(np.float32),
        "w_proj": (rng.standard_normal((192, 192)) * 0.02).astype(np.float32),
        "b_proj": np.zeros((192,), np.float32),
        "cw0": (rng.standard_normal((64, 1, 3, 3)) * 0.1).astype(np.float32),
        "cb0": np.zeros((64,), np.float32),
        "cw1": (rng.standard_normal((64, 1, 3, 3)) * 0.1).astype(np.float32),
        "cb1": np.zeros((64,), np.float32),
        "cw2": (rng.standard_normal((64, 1, 3, 3)) * 0.1).astype(np.float32),
        "cb2": np.zeros((64,), np.float32),
        "w_sqkv": (rng.standard_normal((576, 192)) * 0.02).astype(np.float32),
        "w_sproj": (rng.standard_normal((192, 192)) * 0.02).astype(np.float32),
        "b_sproj": np.zeros((192,), np.float32),
    }
    out = kernel(**ins)
    print("out", out.shape, out.dtype, float(np.abs(out).max()))
